# revision 1
# baseline (speedup 1.0000x reference)
"""Trainium2 Bass kernel for nn_CrossAttention_74818330296332.

Reference computation (per batch b):
  q   = Conv1x1(x, Wq)                          # [C, H, W]
  kv  = Conv3x3_same(condition, Wkv) + bkv      # [2C, H, W]
  k, v = split(kv)                              # each [C, H, W]
  S   = q @ k^T over W (per channel)            # [C, H, H]
  A   = softmax(S * C**-0.5, axis=-1)
  att = A @ v                                   # [C, H, W]
  out = Conv1x1(att, Wproj) + bproj + x

Sharding: data-parallel over batch B=8 across the 8 NeuronCores.

Per-core pipeline (all matmuls bf16 inputs, fp32 PSUM accumulate):
  Phase B (fused with q conv): 3x3 conv as 9 shifted 1x1 matmuls
    accumulated in PSUM; spatial chunks of 32 rows with 1-row halo and
    zero-padded columns held in persistent A/B SBUF tiles. Writes
    q/k/v [C, H*W] bf16 to DRAM scratch.
  Phase C (attention, per channel): Q^T/K^T loaded via DMA-transpose
    (so score matmuls need no on-chip transposes); S^T = K Q^T computed
    with g on partitions; exp via ACT (no max subtraction -- logits are
    provably < 2 in magnitude for this problem's scale); softmax
    denominator via ones-matmul over partitions; normalization folded
    into the PSUM evacuation as a per-partition scalar multiply.
  Phase D: 1x1 proj conv + bias + fp32 residual add, streaming chunks.
"""

import os
import sys
import types

import numpy as np
import ml_dtypes

# Make NTFF tracing available if requested (no-op for plain runs).
try:
    import antenv

    if not hasattr(antenv, "axon_hooks"):
        _m = types.ModuleType("antenv.axon_hooks")
        _hook = [None]
        _m.set_axon_ntff_profile_hook = lambda h: _hook.__setitem__(0, h)
        _m.get_axon_ntff_profile_hook = lambda: _hook[0]
        sys.modules["antenv.axon_hooks"] = _m
        antenv.axon_hooks = _m
except Exception:
    pass

import concourse.bass as bass  # noqa: E402
import concourse.tile as tile  # noqa: E402
from concourse import bacc, mybir  # noqa: E402
from concourse.bass_utils import run_bass_kernel_spmd  # noqa: E402

BF16 = mybir.dt.bfloat16
F32 = mybir.dt.float32
AFT = mybir.ActivationFunctionType

B, C, C_COND, H, W = 8, 128, 256, 256, 256
HW = H * W
SCALE = float(C) ** -0.5

N_CHUNKS = 8          # phase B spatial chunks
RC = H // N_CHUNKS    # rows per chunk (32)
N_GROUPS = 16         # phase C channel groups
GC = C // N_GROUPS    # channels per group (8)
D_CHUNK = 1024        # phase D pixels per chunk


def _emit(tc):
    nc = tc.nc

    x_d = nc.dram_tensor("x", [C, HW], F32, kind="ExternalInput").ap()
    cond_d = nc.dram_tensor("cond", [C_COND, H, W], F32, kind="ExternalInput").ap()
    wq_d = nc.dram_tensor("wq", [C, C], BF16, kind="ExternalInput").ap()
    wkv_d = nc.dram_tensor("wkv", [36, 128, 128], BF16, kind="ExternalInput").ap()
    bkv_d = nc.dram_tensor("bkv", [128, 2], F32, kind="ExternalInput").ap()
    wproj_d = nc.dram_tensor("wproj", [C, C], BF16, kind="ExternalInput").ap()
    bproj_d = nc.dram_tensor("bproj", [C, 1], F32, kind="ExternalInput").ap()

    q_d = nc.dram_tensor("q_s", [C, HW], BF16, kind="Internal").ap()
    k_d = nc.dram_tensor("k_s", [C, HW], BF16, kind="Internal").ap()
    v_d = nc.dram_tensor("v_s", [C, HW], BF16, kind="Internal").ap()
    att_d = nc.dram_tensor("att_s", [C, HW], BF16, kind="Internal").ap()
    out_d = nc.dram_tensor("out", [C, HW], F32, kind="ExternalOutput").ap()

    # ---------------- globals ----------------
    with tc.tile_pool(name="glob", bufs=1) as glob:
        ones_sb = glob.tile([128, 1], BF16)
        nc.vector.memset(ones_sb, 1.0)
        wproj_sb = glob.tile([128, 128], BF16)
        nc.sync.dma_start(wproj_sb[:], wproj_d[:])
        bproj_sb = glob.tile([128, 1], F32)
        nc.sync.dma_start(bproj_sb[:], bproj_d[:])

        # ---------------- phase B: q conv + kv conv ----------------
        with tc.tile_pool(name="pb_const", bufs=1) as pbc, \
             tc.tile_pool(name="pb_kvps", bufs=2, space="PSUM") as kv_psp, \
             tc.tile_pool(name="pb_qps", bufs=2, space="PSUM") as q_psp, \
             tc.tile_pool(name="pb_stage", bufs=3) as stp, \
             tc.tile_pool(name="pb_x", bufs=2) as xp:
            wq_sb = pbc.tile([128, 128], BF16)
            nc.sync.dma_start(wq_sb[:], wq_d[:])
            wkv_sb = pbc.tile([128, 36, 128], BF16)
            nc.sync.dma_start(wkv_sb[:], wkv_d.rearrange("t i o -> i t o"))
            bkv_sb = pbc.tile([128, 2], F32)
            nc.sync.dma_start(bkv_sb[:], bkv_d[:])

            # persistent A/B cond tiles: [128, 34 rows, 258 cols] with
            # zero pad columns 0 and 257 (w padding of the SAME conv)
            ct = [[pbc.tile([128, RC + 2, W + 2], BF16, name=f"ct{p}{ib}")
                   for ib in range(2)] for p in range(2)]
            for p in range(2):
                for ib in range(2):
                    nc.vector.memset(ct[p][ib][:, :, 0:1], 0.0)
                    nc.vector.memset(ct[p][ib][:, :, W + 1:W + 2], 0.0)

            for chunk in range(N_CHUNKS):
                r0 = chunk * RC
                par = chunk % 2
                # load cond rows [r0-1, r0+RC+1) with edge clipping
                for ib in range(2):
                    t = ct[par][ib]
                    lo = r0 - 1
                    hi = r0 + RC + 1
                    tlo = 0
                    if lo < 0:
                        nc.vector.memset(t[:, 0:1, :], 0.0)
                        lo, tlo = 0, 1
                    if hi > H:
                        nc.vector.memset(t[:, RC + 1:RC + 2, :], 0.0)
                        hi = H
                    nc.gpsimd.dma_start(
                        out=t[:, tlo:tlo + (hi - lo), 1:W + 1],
                        in_=cond_d[ib * 128:(ib + 1) * 128, lo:hi, :])

                # kv conv: 8 pairs x 2 subs of 512 px, 2 out blocks
                for pair in range(RC // 4):
                    for ob in range(2):
                        ps = kv_psp.tile([128, 1024], F32, name="kvps")
                        for s in range(2):
                            first = True
                            for dy in range(3):
                                for dx in range(3):
                                    for ib in range(2):
                                        ti = ((ob * 3 + dy) * 3 + dx) * 2 + ib
                                        rr = 4 * pair + 2 * s + dy
                                        nc.tensor.matmul(
                                            ps[:, s * 512:(s + 1) * 512],
                                            lhsT=wkv_sb[:, ti, :],
                                            rhs=ct[par][ib][:, rr:rr + 2, dx:dx + W],
                                            start=first,
                                            stop=(dy == 2 and dx == 2 and ib == 1))
                                        first = False
                        kvst = stp.tile([128, 1024], BF16, name="kvst")
                        nc.scalar.activation(kvst[:], ps[:], func=AFT.Identity,
                                             bias=bkv_sb[:, ob:ob + 1], scale=1.0)
                        dst = k_d if ob == 0 else v_d
                        off = r0 * W + pair * 1024
                        nc.sync.dma_start(dst[:, off:off + 1024], kvst[:])

                # q conv for the same 32 rows, two halves of 16 rows
                for half in range(2):
                    off = (r0 + 16 * half) * W
                    xt = xp.tile([128, 4096], BF16, name="xt")
                    nc.gpsimd.dma_start(out=xt[:], in_=x_d[:, off:off + 4096])
                    qst = stp.tile([128, 4096], BF16, name="qst")
                    for j2 in range(4):
                        qps = q_psp.tile([128, 1024], F32, name="qps")
                        for s in range(2):
                            j = j2 * 2 + s
                            nc.tensor.matmul(
                                qps[:, s * 512:(s + 1) * 512],
                                lhsT=wq_sb[:],
                                rhs=xt[:, j * 512:(j + 1) * 512],
                                start=True, stop=True)
                        nc.vector.tensor_copy(qst[:, j2 * 1024:(j2 + 1) * 1024],
                                              qps[:])
                    nc.sync.dma_start(q_d[:, off:off + 4096], qst[:])

        # ---------------- phase C: per-channel attention ----------------
        q_v = q_d.rearrange("c (h w) -> c h w", h=H)
        k_v = k_d.rearrange("c (h w) -> c h w", h=H)
        v_v = v_d.rearrange("c (h w) -> c h w", h=H)
        att_v = att_d.rearrange("c (h w) -> c h w", h=H)
        with tc.tile_pool(name="pc_in", bufs=2) as pci, \
             tc.tile_pool(name="pc_est", bufs=3) as pce, \
             tc.tile_pool(name="pc_r", bufs=3) as pcr, \
             tc.tile_pool(name="pc_ao", bufs=2) as pao, \
             tc.tile_pool(name="pc_stps", bufs=3, space="PSUM") as stps, \
             tc.tile_pool(name="pc_dps", bufs=2, space="PSUM") as dps, \
             tc.tile_pool(name="pc_aps", bufs=2, space="PSUM") as aps:
            for g in range(N_GROUPS):
                c0 = g * GC
                qt, kt, vt = [], [], []
                for wb in range(2):
                    qtw = pci.tile([128, GC, H], BF16, name=f"qt{wb}")
                    nc.sync.dma_start(
                        out=qtw[:],
                        in_=q_v[c0:c0 + GC, :, wb * 128:(wb + 1) * 128]
                        .rearrange("c h w -> (c h) w"),
                        transpose=True)
                    qt.append(qtw)
                    ktw = pci.tile([128, GC, H], BF16, name=f"kt{wb}")
                    nc.sync.dma_start(
                        out=ktw[:],
                        in_=k_v[c0:c0 + GC, :, wb * 128:(wb + 1) * 128]
                        .rearrange("c h w -> (c h) w"),
                        transpose=True)
                    kt.append(ktw)
                for gb in range(2):
                    vtg = pci.tile([128, GC, W], BF16, name=f"vt{gb}")
                    nc.sync.dma_start(
                        out=vtg[:],
                        in_=v_v[c0:c0 + GC, gb * 128:(gb + 1) * 128, :]
                        .rearrange("c h w -> h c w"))
                    vt.append(vtg)
                ao = [pao.tile([128, GC, W], BF16, name=f"ao{hb}")
                      for hb in range(2)]

                for ci in range(GC):
                    # S^T[g, h] = sum_w k[g, w] q[h, w]; free = (gb, h)
                    st = stps.tile([128, 512], F32, name="stps")
                    for gb in range(2):
                        for wb in range(2):
                            nc.tensor.matmul(
                                st[:, gb * 256:(gb + 1) * 256],
                                lhsT=kt[wb][:, ci, gb * 128:(gb + 1) * 128],
                                rhs=qt[wb][:, ci, :],
                                start=(wb == 0), stop=(wb == 1))
                    est = pce.tile([128, 512], BF16, name="est")
                    nc.scalar.activation(est[:], st[:], func=AFT.Exp, scale=SCALE)
                    # d[h] = sum_g exp(S^T)[g, h]
                    dp = dps.tile([128, 2], F32, name="dp")
                    for hb in range(2):
                        for gb in range(2):
                            nc.tensor.matmul(
                                dp[:, hb:hb + 1],
                                lhsT=est[:, gb * 256 + hb * 128:
                                         gb * 256 + (hb + 1) * 128],
                                rhs=ones_sb[:],
                                start=(gb == 0), stop=(gb == 1))
                    r = pcr.tile([128, 2], F32, name="r")
                    nc.vector.reciprocal(r[:], dp[:])
                    # att[h, w] = r[h] * sum_g exp(S^T)[g, h] v[g, w]
                    for hb in range(2):
                        ap_ = aps.tile([128, 256], F32, name="attps")
                        for gb in range(2):
                            nc.tensor.matmul(
                                ap_[:],
                                lhsT=est[:, gb * 256 + hb * 128:
                                         gb * 256 + (hb + 1) * 128],
                                rhs=vt[gb][:, ci, :],
                                start=(gb == 0), stop=(gb == 1))
                        nc.vector.tensor_scalar_mul(ao[hb][:, ci, :], ap_[:],
                                                    r[:, hb:hb + 1])
                for hb in range(2):
                    nc.sync.dma_start(
                        out=att_v[c0:c0 + GC, hb * 128:(hb + 1) * 128, :]
                        .rearrange("c h w -> h c w"),
                        in_=ao[hb][:])

        # ---------------- phase D: proj conv + bias + residual ----------------
        with tc.tile_pool(name="pd_in", bufs=3) as pdi, \
             tc.tile_pool(name="pd_out", bufs=3) as pdo, \
             tc.tile_pool(name="pd_ps", bufs=2, space="PSUM") as pdp:
            for chunk in range(HW // D_CHUNK):
                off = chunk * D_CHUNK
                ac = pdi.tile([128, D_CHUNK], BF16, name="ac")
                nc.sync.dma_start(ac[:], att_d[:, off:off + D_CHUNK])
                xc = pdi.tile([128, D_CHUNK], F32, name="xc")
                nc.sync.dma_start(xc[:], x_d[:, off:off + D_CHUNK])
                pp = pdp.tile([128, D_CHUNK], F32, name="pp")
                for s in range(2):
                    nc.tensor.matmul(pp[:, s * 512:(s + 1) * 512],
                                     lhsT=wproj_sb[:],
                                     rhs=ac[:, s * 512:(s + 1) * 512],
                                     start=True, stop=True)
                xb = pdo.tile([128, D_CHUNK], F32, name="xb")
                nc.scalar.activation(xb[:], xc[:], func=AFT.Identity,
                                     bias=bproj_sb[:], scale=1.0)
                oc = pdo.tile([128, D_CHUNK], F32, name="oc")
                nc.vector.tensor_add(oc[:], pp[:], xb[:])
                nc.sync.dma_start(out_d[:, off:off + D_CHUNK], oc[:])


_NC_CACHE = [None]
LAST_RESULT = [None]


def _build_nc():
    if _NC_CACHE[0] is None:
        nc = bacc.Bacc("TRN2", target_bir_lowering=False, debug=False,
                       num_devices=8)
        with tile.TileContext(nc) as tc:
            _emit(tc)
        nc.compile()
        _NC_CACHE[0] = nc
    return _NC_CACHE[0]


def kernel(x, condition, Wq, Wkv, bkv, Wproj, bproj):
    x = np.asarray(x, dtype=np.float32)
    condition = np.asarray(condition, dtype=np.float32)
    Wq = np.asarray(Wq, dtype=np.float32)
    Wkv = np.asarray(Wkv, dtype=np.float32)
    bkv = np.asarray(bkv, dtype=np.float32)
    Wproj = np.asarray(Wproj, dtype=np.float32)
    bproj = np.asarray(bproj, dtype=np.float32)

    bf = ml_dtypes.bfloat16
    wq_h = np.ascontiguousarray(Wq[:, :, 0, 0].T).astype(bf)
    # [ob, o, ib, i, dy, dx] -> [ob, dy, dx, ib, i, o] -> [36, 128, 128]
    wkv_h = np.ascontiguousarray(
        Wkv.reshape(2, 128, 2, 128, 3, 3).transpose(0, 4, 5, 2, 3, 1)
    ).reshape(36, 128, 128).astype(bf)
    bkv_h = np.ascontiguousarray(bkv.reshape(2, 128).T)
    wproj_h = np.ascontiguousarray(Wproj[:, :, 0, 0].T).astype(bf)
    bproj_h = np.ascontiguousarray(bproj.reshape(C, 1))

    in_maps = []
    for b in range(B):
        in_maps.append({
            "x": np.ascontiguousarray(x[b].reshape(C, HW)),
            "cond": np.ascontiguousarray(condition[b]),
            "wq": wq_h,
            "wkv": wkv_h,
            "bkv": bkv_h,
            "wproj": wproj_h,
            "bproj": bproj_h,
        })

    nc = _build_nc()
    res = run_bass_kernel_spmd(nc, in_maps, core_ids=list(range(B)))
    LAST_RESULT[0] = res
    out = np.stack([np.asarray(res.results[b]["out"], dtype=np.float32)
                    for b in range(B)])
    return out.reshape(B, C, H, W)


# revision 5
# speedup vs baseline: 1.4578x; 1.4578x over previous
"""Trainium2 Bass kernel for nn_CrossAttention_74818330296332.

Reference computation (per batch b):
  q   = Conv1x1(x, Wq)                          # [C, H, W]
  kv  = Conv3x3_same(condition, Wkv) + bkv      # [2C, H, W]
  k, v = split(kv)                              # each [C, H, W]
  S   = q @ k^T over W (per channel)            # [C, H, H]
  A   = softmax(S * C**-0.5, axis=-1)
  att = A @ v                                   # [C, H, W]
  out = Conv1x1(att, Wproj) + bproj + x

Sharding: data-parallel over batch B=8 across the 8 NeuronCores.

Per-core pipeline (all matmuls bf16 inputs, fp32 PSUM accumulate):
  Phase B (fused with q conv): 3x3 conv as 9 shifted 1x1 matmuls
    accumulated in PSUM; spatial chunks of 32 rows with 1-row halo and
    zero-padded columns held in persistent A/B SBUF tiles. Writes
    q/k/v [C, H*W] bf16 to DRAM scratch.
  Phase C (attention, per channel): Q^T/K^T loaded via DMA-transpose
    (so score matmuls need no on-chip transposes); S^T = K Q^T computed
    with g on partitions; exp via ACT (no max subtraction -- logits are
    provably < 2 in magnitude for this problem's scale); softmax
    denominator via ones-matmul over partitions; normalization folded
    into the PSUM evacuation as a per-partition scalar multiply.
  Phase D: 1x1 proj conv + bias + fp32 residual add, streaming chunks.
"""

import os
import sys
import types

import numpy as np
import ml_dtypes

# Make NTFF tracing available if requested (no-op for plain runs).
try:
    import antenv

    if not hasattr(antenv, "axon_hooks"):
        _m = types.ModuleType("antenv.axon_hooks")
        _hook = [None]
        _m.set_axon_ntff_profile_hook = lambda h: _hook.__setitem__(0, h)
        _m.get_axon_ntff_profile_hook = lambda: _hook[0]
        sys.modules["antenv.axon_hooks"] = _m
        antenv.axon_hooks = _m
except Exception:
    pass

import concourse.bass as bass  # noqa: E402
import concourse.tile as tile  # noqa: E402
from concourse import bacc, mybir  # noqa: E402
from concourse.bass_utils import run_bass_kernel_spmd  # noqa: E402

BF16 = mybir.dt.bfloat16
F32 = mybir.dt.float32
FP8 = mybir.dt.float8e4
PM = mybir.MatmulPerfMode
AFT = mybir.ActivationFunctionType

B, C, C_COND, H, W = 8, 128, 256, 256, 256
HW = H * W
SCALE = float(C) ** -0.5
WKV_SCALE = 64.0      # fp8 weight pre-scale (undone in the PSUM evacuation)

N_CHUNKS = 8          # phase B spatial chunks
RC = H // N_CHUNKS    # rows per chunk (32)
CW = W + 16           # padded row length (272; %16 for DoubleRow AP strides)
N_GROUPS = 16         # phase C channel groups
GC = C // N_GROUPS    # channels per group (8)
D_CHUNK = 1024        # phase D pixels per chunk


def _emit(tc):
    nc = tc.nc

    x_d = nc.dram_tensor("x", [C, HW], F32, kind="ExternalInput").ap()
    cond_d = nc.dram_tensor("cond", [C_COND, H, W], F32, kind="ExternalInput").ap()
    wq_d = nc.dram_tensor("wq", [C, C], BF16, kind="ExternalInput").ap()
    wkv_d = nc.dram_tensor("wkv", [128, 18, 2, 128], FP8, kind="ExternalInput").ap()
    bkv_d = nc.dram_tensor("bkv", [128, 2], F32, kind="ExternalInput").ap()
    wproj_d = nc.dram_tensor("wproj", [C, C], BF16, kind="ExternalInput").ap()
    bproj_d = nc.dram_tensor("bproj", [C, 1], F32, kind="ExternalInput").ap()

    q_d = nc.dram_tensor("q_s", [C, HW], BF16, kind="Internal").ap()
    k_d = nc.dram_tensor("k_s", [C, HW], BF16, kind="Internal").ap()
    v_d = nc.dram_tensor("v_s", [C, HW], BF16, kind="Internal").ap()
    att_d = nc.dram_tensor("att_s", [C, HW], BF16, kind="Internal").ap()
    out_d = nc.dram_tensor("out", [C, HW], F32, kind="ExternalOutput").ap()

    # ---------------- globals ----------------
    with tc.tile_pool(name="glob", bufs=1) as glob:
        ones_sb = glob.tile([128, 1], BF16)
        nc.vector.memset(ones_sb, 1.0)
        wproj_sb = glob.tile([128, 128], BF16)
        nc.sync.dma_start(wproj_sb[:], wproj_d[:])
        bproj_sb = glob.tile([128, 1], F32)
        nc.sync.dma_start(bproj_sb[:], bproj_d[:])

        # ---------------- phase B: q conv (bf16) + kv conv (fp8 DoubleRow) ----
        with tc.tile_pool(name="pb_const", bufs=1) as pbc, \
             tc.tile_pool(name="pb_ps", bufs=2, space="PSUM") as cvp, \
             tc.tile_pool(name="pb_stage", bufs=3) as stp, \
             tc.tile_pool(name="pb_x", bufs=2) as xp:
            wq_sb = pbc.tile([128, 128], BF16)
            nc.sync.dma_start(wq_sb[:], wq_d[:])
            # [i, t=(ob,dy,dx), kt=ib, o] fp8, pre-scaled by WKV_SCALE
            wkv_sb = pbc.tile([128, 18, 2, 128], FP8)
            nc.sync.dma_start(wkv_sb[:], wkv_d[:])
            bkv_sb = pbc.tile([128, 2], F32)
            nc.sync.dma_start(bkv_sb[:], bkv_d[:])

            # persistent A/B cond tiles: [128, kt=ib, 34 rows, 272 cols] fp8
            # with zero pad columns 0 and 257.. (w padding of the SAME conv)
            ct = [pbc.tile([128, 2, RC + 2, CW], FP8, name=f"ct{p}")
                  for p in range(2)]
            for p in range(2):
                nc.vector.memset(ct[p][:, :, :, 0:1], 0.0)
                nc.vector.memset(ct[p][:, :, :, W + 1:CW], 0.0)

            for chunk in range(N_CHUNKS):
                r0 = chunk * RC
                par = chunk % 2
                t = ct[par]
                # load cond rows [r0-1, r0+RC+1) with edge clipping
                lo = r0 - 1
                hi = r0 + RC + 1
                tlo = 0
                if lo < 0:
                    nc.vector.memset(t[:, :, 0:1, :], 0.0)
                    lo, tlo = 0, 1
                if hi > H:
                    nc.vector.memset(t[:, :, RC + 1:RC + 2, :], 0.0)
                    hi = H
                for ib in range(2):
                    nc.gpsimd.dma_start(
                        out=t[:, ib, tlo:tlo + (hi - lo), 1:W + 1],
                        in_=cond_d[ib * 128:(ib + 1) * 128, lo:hi, :])

                # kv conv: 4 quads of 2048 px; per tap one weight feeds 4 MMs
                for quad in range(4):
                    for ob in range(2):
                        ps = cvp.tile([128, 2048], F32, name=f"cv{ob}",
                                      tag="convps")
                        for dy in range(3):
                            for dx in range(3):
                                ti = ob * 9 + dy * 3 + dx
                                for s in range(4):
                                    rr = 8 * quad + 2 * s + dy
                                    nc.tensor.matmul(
                                        ps[:, s * 512:(s + 1) * 512],
                                        lhsT=wkv_sb[:, ti, :, :],
                                        rhs=t[:, :, rr:rr + 2, dx:dx + W],
                                        start=(dy == 0 and dx == 0),
                                        stop=(dy == 2 and dx == 2),
                                        perf_mode=PM.DoubleRow,
                                        skip_group_check=True)
                        kvst = stp.tile([128, 2048], BF16, name="kvst")
                        nc.scalar.activation(kvst[:], ps[:], func=AFT.Identity,
                                             bias=bkv_sb[:, ob:ob + 1],
                                             scale=1.0 / WKV_SCALE)
                        dst = k_d if ob == 0 else v_d
                        off = r0 * W + quad * 2048
                        nc.scalar.dma_start(dst[:, off:off + 2048], kvst[:])

                # q conv for the same 32 rows, two halves of 16 rows
                for half in range(2):
                    off = (r0 + 16 * half) * W
                    xt = xp.tile([128, 4096], BF16, name="xt")
                    nc.gpsimd.dma_start(out=xt[:], in_=x_d[:, off:off + 4096])
                    qst = stp.tile([128, 4096], BF16, name="qst")
                    for j2 in range(2):
                        qps = cvp.tile([128, 2048], F32, name="qps",
                                       tag="convps")
                        for s in range(4):
                            j = j2 * 4 + s
                            nc.tensor.matmul(
                                qps[:, s * 512:(s + 1) * 512],
                                lhsT=wq_sb[:],
                                rhs=xt[:, j * 512:(j + 1) * 512],
                                start=True, stop=True)
                        nc.vector.tensor_copy(qst[:, j2 * 2048:(j2 + 1) * 2048],
                                              qps[:])
                    nc.sync.dma_start(q_d[:, off:off + 4096], qst[:])

        # ---------------- phase C: per-channel attention ----------------
        q_v = q_d.rearrange("c (h w) -> c h w", h=H)
        k_v = k_d.rearrange("c (h w) -> c h w", h=H)
        v_v = v_d.rearrange("c (h w) -> c h w", h=H)
        att_v = att_d.rearrange("c (h w) -> c h w", h=H)
        with tc.tile_pool(name="pc_in", bufs=3) as pci, \
             tc.tile_pool(name="pc_est", bufs=3) as pce, \
             tc.tile_pool(name="pc_r", bufs=3) as pcr, \
             tc.tile_pool(name="pc_ao", bufs=2) as pao, \
             tc.tile_pool(name="pc_stps", bufs=3, space="PSUM") as stps, \
             tc.tile_pool(name="pc_dps", bufs=2, space="PSUM") as dps, \
             tc.tile_pool(name="pc_aps", bufs=2, space="PSUM") as aps:
            for g in range(N_GROUPS):
                c0 = g * GC
                qt, kt, vt = [], [], []
                for wb in range(2):
                    qtw = pci.tile([128, GC, H], BF16, name=f"qt{wb}")
                    nc.sync.dma_start(
                        out=qtw[:],
                        in_=q_v[c0:c0 + GC, :, wb * 128:(wb + 1) * 128]
                        .rearrange("c h w -> (c h) w"),
                        transpose=True)
                    qt.append(qtw)
                    ktw = pci.tile([128, GC, H], BF16, name=f"kt{wb}")
                    nc.sync.dma_start(
                        out=ktw[:],
                        in_=k_v[c0:c0 + GC, :, wb * 128:(wb + 1) * 128]
                        .rearrange("c h w -> (c h) w"),
                        transpose=True)
                    kt.append(ktw)
                for gb in range(2):
                    vtg = pci.tile([128, GC, W], BF16, name=f"vt{gb}")
                    nc.gpsimd.dma_start(
                        out=vtg[:],
                        in_=v_v[c0:c0 + GC, gb * 128:(gb + 1) * 128, :]
                        .rearrange("c h w -> h c w"))
                    vt.append(vtg)
                ao = [pao.tile([128, GC, W], BF16, name=f"ao{hb}")
                      for hb in range(2)]

                for ci in range(GC):
                    # S^T[g, h] = sum_w k[g, w] q[h, w]; free = (gb, h)
                    st = stps.tile([128, 512], F32, name="stps")
                    for gb in range(2):
                        for wb in range(2):
                            nc.tensor.matmul(
                                st[:, gb * 256:(gb + 1) * 256],
                                lhsT=kt[wb][:, ci, gb * 128:(gb + 1) * 128],
                                rhs=qt[wb][:, ci, :],
                                start=(wb == 0), stop=(wb == 1))
                    est = pce.tile([128, 512], BF16, name="est")
                    nc.scalar.activation(est[:], st[:], func=AFT.Exp, scale=SCALE)
                    # d[h] = sum_g exp(S^T)[g, h]
                    dp = dps.tile([128, 2], F32, name="dp")
                    for hb in range(2):
                        for gb in range(2):
                            nc.tensor.matmul(
                                dp[:, hb:hb + 1],
                                lhsT=est[:, gb * 256 + hb * 128:
                                         gb * 256 + (hb + 1) * 128],
                                rhs=ones_sb[:],
                                start=(gb == 0), stop=(gb == 1))
                    r = pcr.tile([128, 2], F32, name="r")
                    nc.vector.reciprocal(r[:], dp[:])
                    # att[h, w] = r[h] * sum_g exp(S^T)[g, h] v[g, w]
                    for hb in range(2):
                        ap_ = aps.tile([128, 256], F32, name="attps")
                        for gb in range(2):
                            nc.tensor.matmul(
                                ap_[:],
                                lhsT=est[:, gb * 256 + hb * 128:
                                         gb * 256 + (hb + 1) * 128],
                                rhs=vt[gb][:, ci, :],
                                start=(gb == 0), stop=(gb == 1))
                        nc.vector.tensor_scalar_mul(ao[hb][:, ci, :], ap_[:],
                                                    r[:, hb:hb + 1])
                for hb in range(2):
                    nc.scalar.dma_start(
                        out=att_v[c0:c0 + GC, hb * 128:(hb + 1) * 128, :]
                        .rearrange("c h w -> h c w"),
                        in_=ao[hb][:])

        # ---------------- phase D: proj conv + bias + residual ----------------
        with tc.tile_pool(name="pd_in", bufs=3) as pdi, \
             tc.tile_pool(name="pd_out", bufs=3) as pdo, \
             tc.tile_pool(name="pd_ps", bufs=2, space="PSUM") as pdp:
            for chunk in range(HW // D_CHUNK):
                off = chunk * D_CHUNK
                ac = pdi.tile([128, D_CHUNK], BF16, name="ac")
                nc.scalar.dma_start(ac[:], att_d[:, off:off + D_CHUNK])
                xc = pdi.tile([128, D_CHUNK], F32, name="xc")
                nc.sync.dma_start(xc[:], x_d[:, off:off + D_CHUNK])
                pp = pdp.tile([128, D_CHUNK], F32, name="pp")
                for s in range(2):
                    nc.tensor.matmul(pp[:, s * 512:(s + 1) * 512],
                                     lhsT=wproj_sb[:],
                                     rhs=ac[:, s * 512:(s + 1) * 512],
                                     start=True, stop=True)
                xb = pdo.tile([128, D_CHUNK], F32, name="xb")
                nc.scalar.activation(xb[:], xc[:], func=AFT.Identity,
                                     bias=bproj_sb[:], scale=1.0)
                oc = pdo.tile([128, D_CHUNK], F32, name="oc")
                nc.vector.tensor_add(oc[:], pp[:], xb[:])
                nc.gpsimd.dma_start(out_d[:, off:off + D_CHUNK], oc[:])


_NC_CACHE = [None]
LAST_RESULT = [None]


def _build_nc():
    if _NC_CACHE[0] is None:
        nc = bacc.Bacc("TRN2", target_bir_lowering=False, debug=False,
                       num_devices=8)
        with tile.TileContext(nc) as tc:
            _emit(tc)
        nc.compile()
        _NC_CACHE[0] = nc
    return _NC_CACHE[0]


def kernel(x, condition, Wq, Wkv, bkv, Wproj, bproj):
    x = np.asarray(x, dtype=np.float32)
    condition = np.asarray(condition, dtype=np.float32)
    Wq = np.asarray(Wq, dtype=np.float32)
    Wkv = np.asarray(Wkv, dtype=np.float32)
    bkv = np.asarray(bkv, dtype=np.float32)
    Wproj = np.asarray(Wproj, dtype=np.float32)
    bproj = np.asarray(bproj, dtype=np.float32)

    bf = ml_dtypes.bfloat16
    fp8 = ml_dtypes.float8_e4m3
    wq_h = np.ascontiguousarray(Wq[:, :, 0, 0].T).astype(bf)
    # [ob, o, ib, i, dy, dx] -> [i, ob, dy, dx, ib, o] -> [128, 18, 2, 128]
    wkv_h = np.ascontiguousarray(
        (Wkv * WKV_SCALE).reshape(2, 128, 2, 128, 3, 3)
        .transpose(3, 0, 4, 5, 2, 1)
    ).reshape(128, 18, 2, 128).astype(fp8)
    bkv_h = np.ascontiguousarray(bkv.reshape(2, 128).T)
    wproj_h = np.ascontiguousarray(Wproj[:, :, 0, 0].T).astype(bf)
    bproj_h = np.ascontiguousarray(bproj.reshape(C, 1))

    in_maps = []
    for b in range(B):
        in_maps.append({
            "x": np.ascontiguousarray(x[b].reshape(C, HW)),
            "cond": np.ascontiguousarray(condition[b]),
            "wq": wq_h,
            "wkv": wkv_h,
            "bkv": bkv_h,
            "wproj": wproj_h,
            "bproj": bproj_h,
        })

    nc = _build_nc()
    res = run_bass_kernel_spmd(nc, in_maps, core_ids=list(range(B)))
    LAST_RESULT[0] = res
    out = np.stack([np.asarray(res.results[b]["out"], dtype=np.float32)
                    for b in range(B)])
    return out.reshape(B, C, H, W)
